# revision 1
# baseline (speedup 1.0000x reference)
"""GCNConv-style message passing kernel for Trainium2, 8 NeuronCores.

Computes (reference semantics):
    deg  = 1 + segment_sum(edge_weight, col)           # self-loop included
    dinv = deg ** -0.5
    h    = embs @ W
    out[t] = (sum_e norm_e * h[src_e] + dinv[t]^2 * h[t]) * X[t],
             norm_e = dinv[src_e] * ew_e * dinv[t]

Device formulation (matmul commutes past the segment sum):
    embs' = dinv[:, None] * embs                        (host, fp16)
    u[t]  = sum_{e: col=t} ew_e * embs'[src_e] + embs'[t]
    out[t] = (u[t] @ W) * (dinv[t] * X[t])

Sharding: targets split across 8 cores (12500 each). Edges bucketed by
(dest-block of 128 targets, source bank of 25000 rows). Edge source rows are
fetched with dma_gather (int16 bank-local indices); per 128-edge chunk a 0/1
selection matrix S[e, t_loc] = (tloc[e] == iota) is built on DVE and
PE-matmul-accumulated into PSUM u^T[cin, t_loc]. Self loops enter via an
identity matmul of the (contiguous) target rows of embs'.
"""

import numpy as np

import concourse.bacc as bacc
import concourse.tile as tile
from concourse import mybir
from concourse.bass_utils import run_bass_kernel_spmd

P = 128


class _Cfg:
    def __init__(self, n, n_cores, bank_size, sb_group):
        self.N = n
        self.NCORES = n_cores
        self.TPC = n // n_cores              # targets per core
        assert self.TPC * n_cores == n
        self.NSB = -(-self.TPC // P)         # dest blocks of 128 per core
        self.BANK = bank_size                # gather bank rows (int16 < 32768)
        self.NBANK = -(-n // bank_size)
        assert bank_size <= 32768
        self.SB_GROUP = sb_group             # dest blocks per dma_gather


_REAL = _Cfg(n=100000, n_cores=8, bank_size=25000, sb_group=8)


def _host_prep(cfg, X, embs, W, edge_index, edge_weight):
    """Sort/bucket edges, build static chunk schedule + per-core arrays."""
    N, TPC, NSB, BANK, NBANK, NCORES = (
        cfg.N, cfg.TPC, cfg.NSB, cfg.BANK, cfg.NBANK, cfg.NCORES)

    src = np.asarray(edge_index[0], dtype=np.int64)
    col = np.asarray(edge_index[1], dtype=np.int64)
    ew = np.asarray(edge_weight, dtype=np.float64)

    deg = 1.0 + np.bincount(col, weights=ew, minlength=N)
    dinv = (1.0 / np.sqrt(deg)).astype(np.float32)

    embs16 = (dinv[:, None] * np.asarray(embs, np.float32)).astype(np.float16)
    gX = (dinv[:, None] * np.asarray(X, np.float32)).astype(np.float32)

    ew_ones = bool(np.all(np.asarray(edge_weight) == 1.0))

    core = col // TPC
    sb = (col % TPC) // P
    bank = src // BANK
    bucket = (core * NSB + sb) * NBANK + bank
    order = np.argsort(bucket, kind="stable")
    b_sorted = bucket[order]
    src_l = (src[order] - (bank[order] * BANK)).astype(np.int16)
    tl = (col[order] % TPC % P).astype(np.float32)
    ew_s = np.asarray(edge_weight, np.float32)[order]

    counts = np.bincount(bucket, minlength=NCORES * NSB * NBANK)
    counts = counts.reshape(NCORES, NSB, NBANK)
    nch = -(-counts // P)                    # ceil chunks per (core, sb, bank)
    nch = nch.max(axis=0)                    # static across cores [NSB, NBANK]
    nch[:, 0] = np.maximum(nch[:, 0], 1)     # first bucket must init PSUM

    # dest-block groups for gather granularity
    groups = [list(range(g, min(g + cfg.SB_GROUP, NSB)))
              for g in range(0, NSB, cfg.SB_GROUP)]

    # slot layout: for gi, for bank, for sb in group, chunks of (sb, bank)
    chunk_base = np.zeros((NSB, NBANK), np.int64)   # chunk index of bucket
    seg = []                                        # (gi, b) -> (chunk_off, nchunks)
    pos = 0
    for gi, sbs in enumerate(groups):
        for b in range(NBANK):
            off = pos
            for s in sbs:
                chunk_base[s, b] = pos
                pos += nch[s, b]
            seg.append((off, pos - off))
    nch_tot = pos
    slots_tot = nch_tot * P

    # scatter edges into slots
    slot_base = chunk_base * P                       # [NSB, NBANK]
    cnt_flat = counts.reshape(-1)
    starts = np.zeros_like(cnt_flat)
    np.cumsum(cnt_flat[:-1], out=starts[1:])
    rank = np.arange(len(order)) - starts[b_sorted]
    sb_s = (b_sorted // NBANK) % NSB
    bk_s = b_sorted % NBANK
    core_s = b_sorted // (NSB * NBANK)
    dest = slot_base[sb_s, bk_s] + rank

    IDX = np.zeros((NCORES, slots_tot), np.int16)
    TL = np.full((NCORES, slots_tot), -1000.0, np.float32)
    IDX[core_s, dest] = src_l
    TL[core_s, dest] = tl
    EW = None
    if not ew_ones:
        EW = np.ones((NCORES, slots_tot), np.float32)
        EW[core_s, dest] = ew_s

    # pack gather indices: per (gi,b) segment wrap-16, then replicate to 128
    idx_packed = IDX.reshape(NCORES, slots_tot // 16, 16).transpose(0, 2, 1)
    # idx i of a segment must live at [i%16, seg_col_off + i//16]; since
    # segments are slot-aligned to 128 (chunks), per-segment wrapping equals
    # global wrapping restricted to the segment's columns.
    idx_all = np.tile(idx_packed, (1, 8, 1)).astype(np.int16)  # [C,128,slots/16]

    tloc_all = TL.reshape(NCORES, nch_tot, P).transpose(0, 2, 1).copy()
    ew_all = None
    if EW is not None:
        ew_all = EW.reshape(NCORES, nch_tot, P).transpose(0, 2, 1).copy()

    iota = np.tile(np.arange(P, dtype=np.float32), (P, 1))
    ident = np.eye(P, dtype=np.float16)

    sched = dict(groups=groups, nch=nch, chunk_base=chunk_base, seg=seg,
                 nch_tot=nch_tot, ew_ones=ew_ones)
    in_maps = []
    for c in range(NCORES):
        m = dict(
            embs16=embs16,
            w32=np.asarray(W, np.float32),
            gx=np.ascontiguousarray(gX[c * TPC:(c + 1) * TPC]),
            selfrows=np.ascontiguousarray(embs16[c * TPC:(c + 1) * TPC]),
            idxall=np.ascontiguousarray(idx_all[c]),
            tlocall=np.ascontiguousarray(tloc_all[c]),
            iota=iota,
            ident=ident,
        )
        if ew_all is not None:
            m["ewall"] = np.ascontiguousarray(ew_all[c])
        in_maps.append(m)
    return sched, in_maps


def _build_program(cfg, sched):
    N, TPC, NSB, BANK, NBANK = cfg.N, cfg.TPC, cfg.NSB, cfg.BANK, cfg.NBANK
    groups, nch, chunk_base, seg, nch_tot, ew_ones = (
        sched["groups"], sched["nch"], sched["chunk_base"], sched["seg"],
        sched["nch_tot"], sched["ew_ones"])
    slots_tot = nch_tot * P

    nc = bacc.Bacc("TRN2", target_bir_lowering=False, debug=False,
                   num_devices=cfg.NCORES)
    t_embs16 = nc.dram_tensor("embs16", [N, P], mybir.dt.float16,
                              kind="ExternalInput").ap()
    t_w = nc.dram_tensor("w32", [P, P], mybir.dt.float32,
                         kind="ExternalInput").ap()
    t_gx = nc.dram_tensor("gx", [TPC, P], mybir.dt.float32,
                          kind="ExternalInput").ap()
    t_idx = nc.dram_tensor("idxall", [P, slots_tot // 16], mybir.dt.int16,
                           kind="ExternalInput").ap()
    t_tloc = nc.dram_tensor("tlocall", [P, nch_tot], mybir.dt.float32,
                            kind="ExternalInput").ap()
    t_iota = nc.dram_tensor("iota", [P, P], mybir.dt.float32,
                            kind="ExternalInput").ap()
    t_ident = nc.dram_tensor("ident", [P, P], mybir.dt.float16,
                             kind="ExternalInput").ap()
    t_selfrows = nc.dram_tensor("selfrows", [TPC, P], mybir.dt.float16,
                                kind="ExternalInput").ap()
    t_ew = None
    if not ew_ones:
        t_ew = nc.dram_tensor("ewall", [P, nch_tot], mybir.dt.float32,
                              kind="ExternalInput").ap()
    t_out = nc.dram_tensor("out", [TPC, P], mybir.dt.float32,
                           kind="ExternalOutput").ap()

    with tile.TileContext(nc) as tc:
        with tc.tile_pool(name="const", bufs=1) as cpool, \
             tc.tile_pool(name="meta", bufs=1) as mpool, \
             tc.tile_pool(name="gpool", bufs=6) as gpool, \
             tc.tile_pool(name="spool", bufs=6) as spool, \
             tc.tile_pool(name="xfer", bufs=4) as xfer, \
             tc.tile_pool(name="psu", bufs=4, space="PSUM") as psu, \
             tc.tile_pool(name="psb", bufs=4, space="PSUM") as psb:

            iota_t = cpool.tile([P, P], mybir.dt.float32)
            nc.sync.dma_start(out=iota_t, in_=t_iota)
            ident_t = cpool.tile([P, P], mybir.dt.float16)
            nc.sync.dma_start(out=ident_t, in_=t_ident)
            w_t = cpool.tile([P, P], mybir.dt.float32)
            nc.sync.dma_start(out=w_t, in_=t_w)
            idx_t = mpool.tile([P, slots_tot // 16], mybir.dt.int16)
            nc.sync.dma_start(out=idx_t, in_=t_idx)
            tloc_t = mpool.tile([P, nch_tot], mybir.dt.float32)
            nc.sync.dma_start(out=tloc_t, in_=t_tloc)
            ew_t = None
            if t_ew is not None:
                ew_t = mpool.tile([P, nch_tot], mybir.dt.float32)
                nc.sync.dma_start(out=ew_t, in_=t_ew)

            for gi, sbs in enumerate(groups):
                g_tiles = []
                for b in range(NBANK):
                    off, nseg = seg[gi * NBANK + b]
                    if nseg == 0:
                        g_tiles.append(None)
                        continue
                    g_t = gpool.tile([P, nseg, P], mybir.dt.float16, tag="g")
                    rows = min(BANK, N - b * BANK)
                    nc.gpsimd.dma_gather(
                        out_ap=g_t[:, :, :],
                        in_ap=t_embs16[b * BANK: b * BANK + rows, :],
                        idxs_ap=idx_t[:, off * 8:(off + nseg) * 8],
                        num_idxs=nseg * P,
                        num_idxs_reg=nseg * P,
                        elem_size=P,
                        single_packet=False,
                    )
                    g_tiles.append(g_t)

                for s in sbs:
                    t0 = s * P
                    tw = min(P, TPC - t0)
                    psum_u = psu.tile([P, P], mybir.dt.float32, space="PSUM")
                    first = True
                    for b in range(NBANK):
                        off, nseg = seg[gi * NBANK + b]
                        for j in range(int(nch[s, b])):
                            ch = int(chunk_base[s, b]) + j
                            s_t = spool.tile([P, P], mybir.dt.float16, tag="s")
                            nc.vector.tensor_tensor(
                                out=s_t, in0=iota_t,
                                in1=tloc_t[:, ch:ch + 1].to_broadcast([P, P]),
                                op=mybir.AluOpType.is_equal,
                            )
                            if ew_t is not None:
                                s2 = spool.tile([P, P], mybir.dt.float16,
                                                tag="s2")
                                nc.vector.tensor_tensor(
                                    out=s2, in0=s_t,
                                    in1=ew_t[:, ch:ch + 1].to_broadcast([P, P]),
                                    op=mybir.AluOpType.mult,
                                )
                                s_t = s2
                            nc.tensor.matmul(
                                out=psum_u[:, :],
                                lhsT=g_tiles[b][:, ch - off, :],
                                rhs=s_t,
                                start=first, stop=False,
                            )
                            first = False
                    assert not first
                    # self loops: += embs'[t]^T via identity matmul
                    self_t = xfer.tile([P, P], mybir.dt.float16, tag="self")
                    nc.sync.dma_start(
                        out=self_t[:tw, :],
                        in_=t_selfrows[t0:t0 + tw, :],
                    )
                    nc.tensor.matmul(
                        out=psum_u[:, :tw],
                        lhsT=self_t[:tw, :],
                        rhs=ident_t[:tw, :tw],
                        start=False, stop=True,
                    )

                    u_t = xfer.tile([P, P], mybir.dt.float32, tag="u")
                    nc.vector.tensor_copy(out=u_t[:, :tw], in_=psum_u[:, :tw])

                    psum_o = psb.tile([P, P], mybir.dt.float32, space="PSUM")
                    nc.tensor.matmul(out=psum_o[:tw, :], lhsT=u_t[:, :tw],
                                     rhs=w_t, start=True, stop=True)

                    gx_t = xfer.tile([P, P], mybir.dt.float32, tag="gx")
                    nc.sync.dma_start(out=gx_t[:tw, :],
                                      in_=t_gx[t0:t0 + tw, :])
                    o_t = xfer.tile([P, P], mybir.dt.float32, tag="o")
                    nc.vector.tensor_tensor(out=o_t[:tw, :],
                                            in0=psum_o[:tw, :],
                                            in1=gx_t[:tw, :],
                                            op=mybir.AluOpType.mult)
                    nc.sync.dma_start(out=t_out[t0:t0 + tw, :],
                                      in_=o_t[:tw, :])
    nc.compile()
    return nc


def kernel(X, embs, W, edge_index, edge_weight):
    cfg = _REAL
    sched, in_maps = _host_prep(cfg, X, embs, W, edge_index, edge_weight)
    nc = _build_program(cfg, sched)
    res = run_bass_kernel_spmd(nc, in_maps, list(range(cfg.NCORES)))
    out = np.concatenate([res.results[c]["out"] for c in range(cfg.NCORES)],
                         axis=0)
    return out.astype(np.float32)



# revision 2
# speedup vs baseline: 1.4317x; 1.4317x over previous
"""GCNConv-style message passing kernel for Trainium2, 8 NeuronCores.

Reference semantics:
    deg  = 1 + segment_sum(edge_weight, col)
    dinv = deg ** -0.5
    out[t] = ((sum_e norm_e * h[src_e]) + dinv[t]^2 * h[t]) * X[t],
             h = embs @ W,  norm_e = dinv[src_e] * ew_e * dinv[t]

Device formulation (matmul commutes past the segment sum):
    embs' = dinv[:, None] * embs                       (host, fp16)
    u[t]  = sum_{e: col=t} ew_e * embs'[src_e] + embs'[t]
    out[t] = (u[t] @ W) * (dinv[t] * X[t])

Sharding: targets split across 8 cores (12500 each). Edges bucketed by
(source bank of 25000 rows, dest superblock of 1024 targets), padded to
128-slot chunks only at bucket granularity (~5%). Edge source rows are
fetched per bucket with one dma_gather (int16 bank-local indices). Each
chunk of 128 edges is PE-matmul-accumulated into a [128, 512] PSUM tile
(one bank) through a 0/1 selection matrix S[e, tcol]; chunk target
columns are compile-time 128-wide windows (8-aligned) chosen on host
from the union of all 8 cores' targets, so no per-(128-dest-block)
padding is needed. PSUM is pre-zeroed by one wide start=True matmul so
overlapping windows can all accumulate. Self loops enter via a fused
DVE add (psum + embs'^T slice) during the PSUM->SBUF copy. Final
transform: one fp16 matmul with W per 128-target block, multiplied by
dinv*X on DVE.
"""

import numpy as np

import concourse.bacc as bacc
import concourse.tile as tile
from concourse import mybir
from concourse.bass_utils import run_bass_kernel_spmd

P = 128
SB = 1024          # targets per superblock (two PSUM halves of 512)
HALF = 512         # psum tile width (one bank of fp32)
SBATCH = 8         # S-matrices built per DVE instruction
BUFS = dict(gpool=4, spool=16, psu=4, psb=4, upool=3, xfer=4, opool=4)
DEFER_W = True     # slip W-transform into next superblock's stream


class _Cfg:
    def __init__(self, n, n_cores, bank_size):
        self.N = n
        self.NCORES = n_cores
        self.TPC = n // n_cores
        assert self.TPC * n_cores == n
        self.BANK = bank_size
        self.NBANK = -(-n // bank_size)
        assert bank_size <= 32768
        self.NSB = -(-self.TPC // SB)


_REAL = _Cfg(n=100000, n_cores=8, bank_size=25000)


def _host_prep(cfg, X, embs, W, edge_index, edge_weight):
    N, TPC, BANK, NBANK, NCORES, NSB = (
        cfg.N, cfg.TPC, cfg.BANK, cfg.NBANK, cfg.NCORES, cfg.NSB)

    src = np.asarray(edge_index[0], dtype=np.int64)
    col = np.asarray(edge_index[1], dtype=np.int64)
    ew = np.asarray(edge_weight, dtype=np.float64)

    deg = 1.0 + np.bincount(col, weights=ew, minlength=N)
    dinv = (1.0 / np.sqrt(deg)).astype(np.float32)

    embs16 = (dinv[:, None] * np.asarray(embs, np.float32)).astype(np.float16)
    gX = (dinv[:, None] * np.asarray(X, np.float32)).astype(np.float32)
    ew_ones = bool(np.all(np.asarray(edge_weight) == 1.0))

    core = col // TPC
    tloc = col % TPC
    bank = src // BANK
    sb = tloc // SB
    bucket = (core * NBANK + bank) * NSB + sb          # [E]
    order = np.lexsort((tloc, bucket))
    b_s = bucket[order]
    srcl_s = (src[order] - bank[order] * BANK).astype(np.int16)
    tloc_s = tloc[order].astype(np.int32)
    ew_s = np.asarray(edge_weight, np.float32)[order]

    nbuckets = NBANK * NSB
    counts = np.bincount(bucket, minlength=NCORES * nbuckets)
    counts = counts.reshape(NCORES, nbuckets)
    nch = (-(-counts // P)).max(axis=0)                # [nbuckets] shared
    nch = np.maximum(nch, 1)
    chunk_base = np.zeros(nbuckets, np.int64)
    np.cumsum(nch[:-1], out=chunk_base[1:])
    nch_tot = int(nch.sum())
    slots_tot = nch_tot * P

    # scatter edges into slots (bucket-major, rank within own bucket)
    cnt_flat = counts.reshape(-1)
    starts = np.zeros_like(cnt_flat)
    np.cumsum(cnt_flat[:-1], out=starts[1:])
    rank = np.arange(len(order)) - starts[b_s]
    loc_s = b_s % nbuckets
    dest = chunk_base[loc_s] * P + rank
    core_s = b_s // nbuckets

    IDX = np.zeros((NCORES, slots_tot), np.int16)
    TL = np.full((NCORES, slots_tot), -1, np.int32)    # -1 = pad slot
    IDX[core_s, dest] = srcl_s
    TL[core_s, dest] = tloc_s
    EW = None
    if not ew_ones:
        EW = np.ones((NCORES, slots_tot), np.float32)
        EW[core_s, dest] = ew_s

    # --- per-chunk matmul windows from the union of all cores' targets ---
    # windows are 8-aligned, 128 wide, within one 512-target block
    TLc = TL.reshape(NCORES, nch_tot, P)
    # superblock of each chunk (from its bucket)
    chunk_sb = np.zeros(nch_tot, np.int64)
    for bk in range(nbuckets):
        chunk_sb[chunk_base[bk]:chunk_base[bk] + nch[bk]] = bk % NSB
    mm_chunk = []      # chunk id per matmul
    mm_base = []       # global target base of window
    win_of_chunk = []  # list of (start_m, n_m) per chunk
    for q in range(nch_tot):
        vals = TLc[:, q, :]
        vals = np.unique(vals[vals >= 0])
        start_m = len(mm_chunk)
        if len(vals) == 0:
            # fully padded chunk (possible when one core has fewer edges);
            # emit one dead matmul so the schedule stays uniform
            mm_chunk.append(q)
            mm_base.append(int(chunk_sb[q]) * SB)
        else:
            i = 0
            while i < len(vals):
                v = vals[i]
                blk = v // HALF
                base = (v // 8) * 8
                base = min(base, blk * HALF + HALF - P)
                base = max(base, blk * HALF)
                # consume vals in [base, base+P) and same block
                j = i
                while j < len(vals) and vals[j] < base + P \
                        and vals[j] // HALF == blk:
                    j += 1
                mm_chunk.append(q)
                mm_base.append(int(base))
                i = j
        win_of_chunk.append((start_m, len(mm_chunk) - start_m))
    n_mm = len(mm_chunk)
    n_mm_pad = -(-n_mm // SBATCH) * SBATCH
    mm_chunk_a = np.asarray(mm_chunk, np.int64)
    mm_base_a = np.asarray(mm_base, np.int64)

    # per-core TLOCREL [slots per matmul]: rel target or -1000
    TREL = np.full((NCORES, n_mm_pad, P), -1000.0, np.float32)
    EWM = None
    if EW is not None:
        EWM = np.ones((NCORES, n_mm_pad, P), np.float32)
    for q in range(nch_tot):
        s0, nm = win_of_chunk[q]
        t = TLc[:, q, :]                       # [NCORES, P]
        for k in range(nm):
            m = s0 + k
            base = mm_base_a[m]
            lo = base
            hi = base + P
            if k > 0:
                lo = max(lo, mm_base_a[m - 1] + P)
            sel = (t >= lo) & (t < hi)
            TREL[:, m, :] = np.where(sel, (t - base).astype(np.float32),
                                     -1000.0)
            if EWM is not None:
                EWM[:, m, :] = np.where(sel, EW.reshape(NCORES, nch_tot, P)[:, q, :],
                                        1.0)
    # sanity: every real edge is assigned exactly once
    assert int((TREL >= 0).sum()) == int((TL >= 0).sum())

    trel16 = TREL.astype(np.float16)[:, :, :, None].transpose(0, 2, 1, 3)
    # -> [NCORES, P, n_mm_pad, 1]

    # pack gather indices: wrap-16 then replicate to 128 partitions
    idx_packed = IDX.reshape(NCORES, slots_tot // 16, 16).transpose(0, 2, 1)
    idx_all = np.tile(idx_packed, (1, 8, 1)).astype(np.int16)

    iota8 = np.tile(np.arange(P, dtype=np.float16), (P, SBATCH, 1))
    zeros16 = np.zeros((P, HALF), np.float16)

    selfT = np.empty((NCORES, P, TPC), np.float16)
    for c in range(NCORES):
        selfT[c] = embs16[c * TPC:(c + 1) * TPC].T

    # stop flags: last matmul (in emission order) per psum tile (sb, half).
    # emission order == matmul index order (chunks are bucket-major by
    # (bank, sb); all matmuls of superblock sb across its 4 banks are
    # contiguous per bank but interleaved across banks in bank order).
    # Determine for each (sb, half) the max matmul id.
    mm_sb = np.zeros(n_mm, np.int64)
    mm_half = np.zeros(n_mm, np.int64)
    for m in range(n_mm):
        g_t = mm_base_a[m]
        mm_sb[m] = g_t // SB
        mm_half[m] = (g_t % SB) // HALF

    sched = dict(nch=nch, chunk_base=chunk_base, nch_tot=nch_tot,
                 win_of_chunk=win_of_chunk, mm_chunk=mm_chunk_a,
                 mm_base=mm_base_a, mm_sb=mm_sb, mm_half=mm_half,
                 n_mm=n_mm, n_mm_pad=n_mm_pad, ew_ones=ew_ones)

    in_maps = []
    for c in range(NCORES):
        m = dict(
            embs16=embs16,
            w16=np.asarray(W, np.float16),
            gx=np.ascontiguousarray(gX[c * TPC:(c + 1) * TPC]),
            selft=np.ascontiguousarray(selfT[c]),
            idxall=np.ascontiguousarray(idx_all[c]),
            trel=np.ascontiguousarray(trel16[c]),
            iota8=iota8,
            zeros16=zeros16,
        )
        if EWM is not None:
            m["ewm"] = np.ascontiguousarray(
                EWM[c].astype(np.float16).T[:, :, None])
        in_maps.append(m)
    return sched, in_maps


def _build_program(cfg, sched):
    N, TPC, BANK, NBANK, NSB = cfg.N, cfg.TPC, cfg.BANK, cfg.NBANK, cfg.NSB
    nch, chunk_base, nch_tot = sched["nch"], sched["chunk_base"], sched["nch_tot"]
    win_of_chunk = sched["win_of_chunk"]
    mm_base, mm_sb, mm_half = sched["mm_base"], sched["mm_sb"], sched["mm_half"]
    n_mm, n_mm_pad = sched["n_mm"], sched["n_mm_pad"]
    ew_ones = sched["ew_ones"]
    slots_tot = nch_tot * P

    nch_max = int(nch.max())

    # last matmul id per (sb, half) for stop flags
    last_mm = {}
    for m in range(n_mm):
        last_mm[(int(mm_sb[m]), int(mm_half[m]))] = m

    nc = bacc.Bacc("TRN2", target_bir_lowering=False, debug=False,
                   num_devices=cfg.NCORES)
    t_embs16 = nc.dram_tensor("embs16", [N, P], mybir.dt.float16,
                              kind="ExternalInput").ap()
    t_w16 = nc.dram_tensor("w16", [P, P], mybir.dt.float16,
                           kind="ExternalInput").ap()
    t_gx = nc.dram_tensor("gx", [TPC, P], mybir.dt.float32,
                          kind="ExternalInput").ap()
    t_selft = nc.dram_tensor("selft", [P, TPC], mybir.dt.float16,
                             kind="ExternalInput").ap()
    t_idx = nc.dram_tensor("idxall", [P, slots_tot // 16], mybir.dt.int16,
                           kind="ExternalInput").ap()
    t_trel = nc.dram_tensor("trel", [P, n_mm_pad, 1], mybir.dt.float16,
                            kind="ExternalInput").ap()
    t_iota8 = nc.dram_tensor("iota8", [P, SBATCH, P], mybir.dt.float16,
                             kind="ExternalInput").ap()
    t_zeros16 = nc.dram_tensor("zeros16", [P, HALF], mybir.dt.float16,
                               kind="ExternalInput").ap()
    t_ewm = None
    if not ew_ones:
        t_ewm = nc.dram_tensor("ewm", [P, n_mm_pad, 1], mybir.dt.float16,
                               kind="ExternalInput").ap()
    t_out = nc.dram_tensor("out", [TPC, P], mybir.dt.float32,
                           kind="ExternalOutput").ap()

    with tile.TileContext(nc) as tc:
        with tc.tile_pool(name="psu", bufs=BUFS["psu"], space="PSUM") as psu, \
             tc.tile_pool(name="psb", bufs=BUFS["psb"], space="PSUM") as psb, \
             tc.tile_pool(name="const", bufs=1) as cpool, \
             tc.tile_pool(name="meta", bufs=1) as mpool, \
             tc.tile_pool(name="gpool", bufs=BUFS["gpool"]) as gpool, \
             tc.tile_pool(name="spool", bufs=BUFS["spool"]) as spool, \
             tc.tile_pool(name="upool", bufs=BUFS["upool"]) as upool, \
             tc.tile_pool(name="xfer", bufs=BUFS["xfer"]) as xfer, \
             tc.tile_pool(name="opool", bufs=BUFS["opool"]) as opool:

            iota_t = cpool.tile([P, SBATCH, P], mybir.dt.float16)
            nc.sync.dma_start(out=iota_t, in_=t_iota8)
            zeros_t = cpool.tile([P, HALF], mybir.dt.float16)
            nc.sync.dma_start(out=zeros_t, in_=t_zeros16)
            w_t = cpool.tile([P, P], mybir.dt.float16)
            nc.sync.dma_start(out=w_t, in_=t_w16)
            idx_t = mpool.tile([P, slots_tot // 16], mybir.dt.int16)
            nc.sync.dma_start(out=idx_t, in_=t_idx)
            trel_t = mpool.tile([P, n_mm_pad, 1], mybir.dt.float16)
            nc.sync.dma_start(out=trel_t, in_=t_trel)
            ewm_t = None
            if t_ewm is not None:
                ewm_t = mpool.tile([P, n_mm_pad, 1], mybir.dt.float16)
                nc.sync.dma_start(out=ewm_t, in_=t_ewm)

            # pre-materialize num_idxs registers so no RegisterMove lands
            # mid-stream between gathers (value-cache movs otherwise create
            # forward deps on the Pool queue and deadlock the scheduler)
            nidx_regs = {}
            for bk in range(NBANK * NSB):
                v = int(nch[bk]) * P
                if v not in nidx_regs:
                    nidx_regs[v] = nc.gpsimd.to_reg(v)

            # S-batch cache: one tile per SBATCH-group, built on demand
            s_tiles = {}

            def get_s(m):
                b = m // SBATCH
                if b not in s_tiles:
                    s_t = spool.tile([P, SBATCH, P], mybir.dt.float16,
                                     tag="s")
                    nc.vector.tensor_tensor(
                        out=s_t, in0=iota_t,
                        in1=trel_t[:, b * SBATCH:(b + 1) * SBATCH, :]
                        .to_broadcast([P, SBATCH, P]),
                        op=mybir.AluOpType.is_equal)
                    if ewm_t is not None:
                        s2 = spool.tile([P, SBATCH, P], mybir.dt.float16,
                                        tag="s2")
                        nc.vector.tensor_tensor(
                            out=s2, in0=s_t,
                            in1=ewm_t[:, b * SBATCH:(b + 1) * SBATCH, :]
                            .to_broadcast([P, SBATCH, P]),
                            op=mybir.AluOpType.mult)
                        s_t = s2
                    s_tiles[b] = s_t
                return s_tiles[b][:, m % SBATCH, :]

            pend_w = []          # deferred W-transform work for previous sb

            def emit_w_block(u_t, sb, blk_lo, width):
                """Transform + gate + store one 128-target block."""
                t0 = sb * SB + blk_lo
                psum_o = psb.tile([P, P], mybir.dt.float32, space="PSUM")
                nc.tensor.matmul(out=psum_o[:width, :],
                                 lhsT=u_t[:, blk_lo:blk_lo + width],
                                 rhs=w_t, start=True, stop=True)
                gx_t = xfer.tile([P, P], mybir.dt.float32, tag="gx")
                nc.sync.dma_start(out=gx_t[:width, :],
                                  in_=t_gx[t0:t0 + width, :])
                o_t = opool.tile([P, P], mybir.dt.float32, tag="o")
                nc.vector.tensor_tensor(out=o_t[:width, :],
                                        in0=psum_o[:width, :],
                                        in1=gx_t[:width, :],
                                        op=mybir.AluOpType.mult)
                nc.sync.dma_start(out=t_out[t0:t0 + width, :],
                                  in_=o_t[:width, :])

            for sb in range(NSB):
                sb_w = min(SB, TPC - sb * SB)
                halves = [0] if sb_w <= HALF else [0, 1]
                ps = {}
                for h in halves:
                    ps[h] = psu.tile([P, HALF], mybir.dt.float32,
                                     space="PSUM", name="pu", tag="pu")
                    nc.tensor.matmul(out=ps[h][:, :], lhsT=w_t,
                                     rhs=zeros_t, start=True, stop=False)

                first_bank = True
                for bank in range(NBANK):
                    bk = bank * NSB + sb
                    nbch = int(nch[bk])
                    cb = int(chunk_base[bk])
                    g_full = gpool.tile([P, nch_max, P], mybir.dt.float16,
                                        tag="g", name="g_full")
                    g_t = g_full[:, :nbch, :]
                    rows = min(BANK, N - bank * BANK)
                    nc.gpsimd.dma_gather(
                        out_ap=g_t[:, :, :],
                        in_ap=t_embs16[bank * BANK: bank * BANK + rows, :],
                        idxs_ap=idx_t[:, cb * 8:(cb + nbch) * 8],
                        num_idxs=nbch * P,
                        num_idxs_reg=nidx_regs[nbch * P],
                        elem_size=P,
                        single_packet=False,
                    )
                    for j in range(nbch):
                        q = cb + j
                        s0, nm = win_of_chunk[q]
                        for k in range(nm):
                            m = s0 + k
                            h = int(mm_half[m])
                            bcol = int(mm_base[m]) - sb * SB - h * HALF
                            is_last = (last_mm[(sb, h)] == m)
                            nc.tensor.matmul(
                                out=ps[h][:, bcol:bcol + P],
                                lhsT=g_t[:, j, :],
                                rhs=get_s(m),
                                start=False, stop=is_last,
                                skip_group_check=True,
                            )
                    if first_bank and pend_w:
                        # slip previous superblock's transform into the
                        # middle of this one's matmul stream
                        for args in pend_w:
                            emit_w_block(*args)
                        pend_w = []
                    first_bank = False

                # self loop + PSUM -> SBUF (fp16)
                self_t = xfer.tile([P, SB], mybir.dt.float16, tag="self")
                nc.sync.dma_start(out=self_t[:, :sb_w],
                                  in_=t_selft[:, sb * SB:sb * SB + sb_w])
                u_t = upool.tile([P, SB], mybir.dt.float16, tag="u")
                for h in halves:
                    w0 = h * HALF
                    w1 = min(sb_w, w0 + HALF)
                    nc.vector.tensor_tensor(
                        out=u_t[:, w0:w1], in0=ps[h][:, :w1 - w0],
                        in1=self_t[:, w0:w1], op=mybir.AluOpType.add)

                # defer W-transform into next superblock's stream
                blk_lo = 0
                while blk_lo < sb_w:
                    width = min(P, sb_w - blk_lo)
                    pend_w.append((u_t, sb, blk_lo, width))
                    blk_lo += width
                if not DEFER_W:
                    for args in pend_w:
                        emit_w_block(*args)
                    pend_w = []

            for args in pend_w:
                emit_w_block(*args)

    nc.compile()
    return nc


def kernel(X, embs, W, edge_index, edge_weight):
    cfg = _REAL
    sched, in_maps = _host_prep(cfg, X, embs, W, edge_index, edge_weight)
    nc = _build_program(cfg, sched)
    res = run_bass_kernel_spmd(nc, in_maps, list(range(cfg.NCORES)))
    out = np.concatenate([res.results[c]["out"] for c in range(cfg.NCORES)],
                         axis=0)
    return out.astype(np.float32)
